# revision 14
# baseline (speedup 1.0000x reference)
"""Trainium2 Bass kernel for a differentiable-DTW style module.

Math (per batch b):
    dist[i, j] = |score[b, i] - template[j]|              (i, j in [0, 2048))
    path       = softmax(-dist, axis=-1)                  (row-stochastic)
    warped[b]  = path @ feature[b]                        ([2048, 512])
    l2         = 1e-7 * sqrt(sum(warped ** 2))            (global scalar)

Implementation notes:
  - Data-parallel: batch b -> NeuronCore b (B == 8 == n_cores).
  - The softmax row sums are computed EXACTLY on the host with a sorted
    prefix-sum identity:
        sum_j exp(-|s - t_j|) = exp(-s) * sum_{t_j <= s} exp(t_j)
                              + exp(s)  * sum_{t_j >  s} exp(-t_j)
    so the device never needs a partition-axis reduction; it only scales
    the matmul output rows by the reciprocal denominators.
  - The kernel matrix is generated directly in TRANSPOSED layout
    ET[j, i] = exp(-|s_i - t_j|) (template index on partitions), which is
    exactly the lhsT layout the tensor engine wants; feature in natural
    [j, f] layout is the rhs.  out[i, f] = sum_j ET[j, i] * F[j, f].
  - ET is bf16 for the matmul (1 cycle/col vs 2 for fp32); |s - t| is one
    DVE tensor_scalar (subtract then abs_max vs 0), exp is one ScalarE
    activation per [128, 2048] chunk.
  - The scalar l2 is finished on the host from the gathered output.
"""

import numpy as np
import ml_dtypes

B = 8
S = 2048
F = 512
P = 128
NT = S // P  # 16 chunks / output tiles
NCORES = 8
GA = 8  # phase-A output-tile group (== number of PSUM banks)
L2_REG_WEIGHT = 1e-07

_NC_CACHE = {}


def _build_nc():
    import concourse.mybir as mybir
    import concourse.tile as tile
    from concourse import bacc

    f32 = mybir.dt.float32
    bf16 = mybir.dt.bfloat16
    Alu = mybir.AluOpType
    Act = mybir.ActivationFunctionType

    nc = bacc.Bacc(None, target_bir_lowering=False)
    score = nc.dram_tensor("score", [S], f32, kind="ExternalInput")
    negt = nc.dram_tensor("negt", [S], f32, kind="ExternalInput")
    rden = nc.dram_tensor("rden", [S], f32, kind="ExternalInput")
    # exp(s), exp(-s), exp(-t), exp(t) host-precomputed, bf16 (DVE min-trick)
    pexp = nc.dram_tensor("pexp", [S], bf16, kind="ExternalInput")
    pinv = nc.dram_tensor("pinv", [S], bf16, kind="ExternalInput")
    qexp = nc.dram_tensor("qexp", [S], f32, kind="ExternalInput")
    qinv = nc.dram_tensor("qinv", [S], f32, kind="ExternalInput")
    feat = nc.dram_tensor("feature", [S, F], bf16, kind="ExternalInput")
    warped = nc.dram_tensor("warped", [S, F], f32, kind="ExternalOutput")

    # chunks generated on ScalarE (Abs+Exp); the rest on VectorE via
    # E = min(exp(s)exp(-t), exp(-s)exp(t))  (exactly exp(-|s-t|))
    # DVE chunks lead (their broadcast inputs are smaller + no table load).
    ACT_CHUNKS = {1, 3, 5, 7, 9, 11, 13}
    CHUNK_ORDER = list(range(NT))

    with tile.TileContext(nc) as tc:
        with (
            tc.tile_pool(name="const", bufs=1) as cpool,
            tc.tile_pool(name="feat", bufs=NT) as fpool,
            tc.tile_pool(name="et", bufs=NT) as epool,
            tc.tile_pool(name="dtile", bufs=3) as dpool,
            tc.tile_pool(name="otile", bufs=4) as opool,
            tc.tile_pool(name="ps", bufs=8, space="PSUM") as pspool,
        ):
            # Feature tiles on the SP HWDGE ring, issued first so matmuls can
            # start immediately; broadcasts + consts go on the ACT HWDGE ring
            # so the two streams don't serialize behind each other.
            fts = []
            for c in range(NT):
                ft = fpool.tile([P, F], bf16, tag="ft")
                nc.sync.dma_start(out=ft[:], in_=feat[c * P : (c + 1) * P, :])
                fts.append(ft)

            # broadcast exp(s)/exp(-s) rows (bf16) + per-chunk exp(-t)/exp(t)
            p_bc = cpool.tile([P, S], bf16, tag="pbc")
            nc.scalar.dma_start(
                out=p_bc[:], in_=pexp[:].unsqueeze(0).to_broadcast([P, S])
            )
            pi_bc = cpool.tile([P, S], bf16, tag="pibc")
            nc.scalar.dma_start(
                out=pi_bc[:], in_=pinv[:].unsqueeze(0).to_broadcast([P, S])
            )
            q_sb = cpool.tile([P, NT], f32, tag="qsb")
            nc.scalar.dma_start(out=q_sb[:], in_=qexp[:].rearrange("(c p) -> p c", p=P))
            qi_sb = cpool.tile([P, NT], f32, tag="qisb")
            nc.scalar.dma_start(
                out=qi_sb[:], in_=qinv[:].rearrange("(c p) -> p c", p=P)
            )
            # r_sb[p, m] = 1/den[m*128 + p]
            r_sb = cpool.tile([P, NT], f32, tag="rsb")
            nc.scalar.dma_start(out=r_sb[:], in_=rden[:].rearrange("(m p) -> p m", p=P))
            # score broadcast to all partitions: s_bcast[p, i] = s_i
            s_bcast = cpool.tile([P, S], f32, tag="sb")
            nc.scalar.dma_start(
                out=s_bcast[:], in_=score[:].unsqueeze(0).to_broadcast([P, S])
            )
            # nt_sb[p, c] = -t[c*128 + p]
            nt_sb = cpool.tile([P, NT], f32, tag="tsb")
            nc.scalar.dma_start(
                out=nt_sb[:], in_=negt[:].rearrange("(c p) -> p c", p=P)
            )

            # PE warmup: ~10 throwaway matmuls as soon as the first feature
            # tile lands, so HAM un-throttles before the real work.
            wps = pspool.tile([P, F], f32, tag="ps", name="warmup_ps")
            for _ in range(10):
                nc.tensor.matmul(wps[:], fts[0][:, 0:P], fts[0][:], start=True, stop=True)

            def epilogue(ps, m):
                o = opool.tile([P, F], f32, tag="o")
                nc.vector.tensor_scalar_mul(o[:], ps[:], r_sb[:, m : m + 1])
                nc.sync.dma_start(out=warped[m * P : (m + 1) * P, :], in_=o[:])

            # Phase A: generate ET chunk-by-chunk; as each chunk lands, run
            # the 8 matmuls of output tiles 0..7 (one PSUM bank each).
            psA = [
                pspool.tile([P, F], f32, tag="ps", name=f"psA{m}") for m in range(GA)
            ]
            ets = []
            for c in range(NT):
                et = epool.tile([P, S], bf16, tag="et")
                if c in ACT_CHUNKS:
                    d = dpool.tile([P, S], f32, tag="d")
                    # d[p, i] = |s_i - t_{c*128+p}|  (ScalarE: Abs(in*1 + bias))
                    nc.scalar.activation(
                        out=d[:],
                        in_=s_bcast[:],
                        func=Act.Abs,
                        bias=nt_sb[:, c : c + 1],
                        scale=1.0,
                    )
                    nc.scalar.activation(out=et[:], in_=d[:], func=Act.Exp, scale=-1.0)
                else:
                    m1 = dpool.tile([P, S], bf16, tag="m1")
                    m2 = dpool.tile([P, S], bf16, tag="m2")
                    nc.vector.tensor_scalar_mul(m1[:], p_bc[:], q_sb[:, c : c + 1])
                    nc.vector.tensor_scalar_mul(m2[:], pi_bc[:], qi_sb[:, c : c + 1])
                    nc.vector.tensor_tensor(
                        out=et[:], in0=m1[:], in1=m2[:], op=Alu.min
                    )
                ets.append(et)
                for m in range(GA):
                    nc.tensor.matmul(
                        psA[m][:],
                        et[:, m * P : (m + 1) * P],
                        fts[c][:],
                        start=(c == 0),
                        stop=(c == NT - 1),
                    )
            for m in range(GA):
                epilogue(psA[m], m)

            # Phase B: all ET chunks resident; dense matmuls for tiles 8..15.
            for m in range(GA, NT):
                ps = pspool.tile([P, F], f32, tag="ps", name=f"psB{m}")
                for c in range(NT):
                    nc.tensor.matmul(
                        ps[:],
                        ets[c][:, m * P : (m + 1) * P],
                        fts[c][:],
                        start=(c == 0),
                        stop=(c == NT - 1),
                    )
                epilogue(ps, m)

    nc.compile()
    return nc


def get_nc():
    if "nc" not in _NC_CACHE:
        _NC_CACHE["nc"] = _build_nc()
    return _NC_CACHE["nc"]


def _host_rden(score, template):
    """Exact softmax denominators: rden[b, i] = 1 / sum_j exp(-|s_bi - t_j|)."""
    s = score[:, :, 0].astype(np.float64)  # [B, S]
    t = np.sort(template[0, :, 0].astype(np.float64))  # [S]
    C = np.concatenate([[0.0], np.cumsum(np.exp(t))])  # C[k] = sum_{j<k} e^{t_j}
    D = np.concatenate([[0.0], np.cumsum(np.exp(-t)[::-1])])[::-1]  # sum_{j>=k} e^{-t_j}
    k = np.searchsorted(t, s.ravel(), side="right").reshape(s.shape)
    den = np.exp(-s) * C[k] + np.exp(s) * D[k]
    return (1.0 / den).astype(np.float32)  # [B, S]


def make_in_maps(score, feature, template):
    rden = _host_rden(score, template)
    s = np.ascontiguousarray(score[:, :, 0], dtype=np.float32)  # [B, S]
    t = np.ascontiguousarray(template[0, :, 0], dtype=np.float32)  # [S]
    bf = ml_dtypes.bfloat16
    qexp = np.exp(-t.astype(np.float64)).astype(np.float32)
    qinv = np.exp(t.astype(np.float64)).astype(np.float32)
    in_maps = []
    for b in range(B):
        in_maps.append(
            {
                "score": s[b],
                "negt": -t,
                "rden": np.ascontiguousarray(rden[b]),
                "pexp": np.exp(s[b].astype(np.float64)).astype(bf),
                "pinv": np.exp(-s[b].astype(np.float64)).astype(bf),
                "qexp": qexp,
                "qinv": qinv,
                "feature": np.asarray(feature[b], dtype=np.float32).astype(bf),
            }
        )
    return in_maps


def postprocess(results):
    """results: per-core list of {name: np.ndarray} -> (warped, l2)."""
    warped = np.stack(
        [np.asarray(results[b]["warped"], dtype=np.float32) for b in range(B)]
    )
    l2 = np.float32(
        L2_REG_WEIGHT * np.sqrt(np.sum(warped.astype(np.float64) ** 2))
    )
    return warped, l2


def kernel(score, feature, template):
    from concourse.bass_utils import run_bass_kernel_spmd

    nc = get_nc()
    in_maps = make_in_maps(score, feature, template)
    res = run_bass_kernel_spmd(nc, in_maps, core_ids=list(range(NCORES)))
    return postprocess(res.results)


# revision 16
# speedup vs baseline: 1.0384x; 1.0384x over previous
"""Trainium2 Bass kernel for a differentiable-DTW style module.

Math (per batch b):
    dist[i, j] = |score[b, i] - template[j]|              (i, j in [0, 2048))
    path       = softmax(-dist, axis=-1)                  (row-stochastic)
    warped[b]  = path @ feature[b]                        ([2048, 512])
    l2         = 1e-7 * sqrt(sum over all b of warped ** 2)

Implementation notes:
  - Data-parallel: batch b -> NeuronCore b (B == 8 == n_cores).
  - Softmax row sums are computed EXACTLY on the host with a sorted
    prefix-sum identity:
        sum_j exp(-|s - t_j|) = exp(-s) * sum_{t_j <= s} exp(t_j)
                              + exp(s)  * sum_{t_j >  s} exp(-t_j)
    so the device only scales matmul output rows by 1/den.
  - Kernel matrix generated directly in TRANSPOSED layout
    ET[j, i] = exp(-|s_i - t_j|) (template on partitions) == the lhsT the
    tensor engine wants; feature [j, f] is the rhs.
  - ET is bf16; generation is split between ScalarE (Abs then Exp, with a
    per-partition -t bias) and VectorE (E = min(e^s e^-t, e^-s e^t), two
    tensor_scalar_mul + one tensor_tensor min, all bf16 fast modes).
  - score row is broadcast to 128 partitions with a tensor-engine outer
    product against ones (doubles as HAM warmup); exp(+-s) broadcasts are
    then one ScalarE Exp each.
  - DMAs are coalesced (one feature load, one packed-constant load, four
    grouped stores) because each dma_start costs ~1.2us of sequencer time.
  - Output leaves the device in bf16; host upcasts and finishes the l2.
"""

import numpy as np
import ml_dtypes

B = 8
S = 2048
F = 512
P = 128
NT = S // P  # 16 chunks / output tiles
NCORES = 8
GA = 8  # phase-A output-tile group (== number of PSUM banks)
OG = 4  # output m-tiles grouped per store DMA
L2_REG_WEIGHT = 1e-07

# packed[:, c] column map: 0:16 -t | 16:32 1/den | 32:48 e^-t | 48:64 e^t
C_NT, C_R, C_Q, C_QI = 0, NT, 2 * NT, 3 * NT

_NC_CACHE = {}


def _build_nc():
    import concourse.mybir as mybir
    import concourse.tile as tile
    from concourse import bacc

    f32 = mybir.dt.float32
    bf16 = mybir.dt.bfloat16
    Alu = mybir.AluOpType
    Act = mybir.ActivationFunctionType

    nc = bacc.Bacc(None, target_bir_lowering=False)
    score = nc.dram_tensor("score", [S], f32, kind="ExternalInput")
    packed = nc.dram_tensor("packed", [P, 4 * NT], f32, kind="ExternalInput")
    feat = nc.dram_tensor("feature", [S, F], bf16, kind="ExternalInput")
    warped = nc.dram_tensor("warped", [S, F], bf16, kind="ExternalOutput")

    # ScalarE-generated chunks (Abs+Exp); the rest on VectorE via
    # E = min(exp(s)exp(-t), exp(-s)exp(t))  (exactly exp(-|s-t|))
    ACT_CHUNKS = {0, 2, 4, 6, 8, 10, 12}

    with tile.TileContext(nc) as tc:
        with (
            tc.tile_pool(name="const", bufs=1) as cpool,
            tc.tile_pool(name="feat", bufs=1) as fpool,
            tc.tile_pool(name="et", bufs=NT) as epool,
            tc.tile_pool(name="dtile", bufs=3) as dpool,
            tc.tile_pool(name="otile", bufs=2) as opool,
            tc.tile_pool(name="ps", bufs=8, space="PSUM") as pspool,
        ):
            ones = cpool.tile([1, P], f32, tag="ones")
            nc.gpsimd.memset(ones[:], 1.0)
            scratch = cpool.tile([1, 1], f32, tag="scratch")
            # dummy activation: forces the ACT table set load at t~0
            nc.scalar.activation(out=scratch[:], in_=ones[:, 0:1], func=Act.Exp)

            s_row = cpool.tile([1, S], f32, tag="srow")
            nc.sync.dma_start(out=s_row[:], in_=score[:].unsqueeze(0))
            # one coalesced feature load: ft_all[p, c*F+f] = feature[c*128+p, f]
            ft_all = fpool.tile([P, NT * F], bf16, tag="ft")
            nc.sync.dma_start(
                out=ft_all[:].rearrange("p (c f) -> p c f", c=NT),
                in_=feat[:].rearrange("(c p) f -> p c f", p=P),
            )
            pk = cpool.tile([P, 4 * NT], f32, tag="pk")
            nc.scalar.dma_start(out=pk[:], in_=packed[:])

            def ftc(c):
                return ft_all[:, c * F : (c + 1) * F]

            # Broadcast score row to all partitions via PE outer product with
            # ones (also HAM warmup), then copy PSUM->SBUF on ScalarE.
            s_bcast = cpool.tile([P, S], f32, tag="sb")
            bcps = []
            for h in range(4):
                bp = pspool.tile([P, F], f32, tag="ps", name=f"bcps{h}")
                nc.tensor.matmul(
                    bp[:], ones[:], s_row[:, h * F : (h + 1) * F], start=True, stop=True
                )
                bcps.append(bp)
            for h in range(4):
                nc.scalar.copy(out=s_bcast[:, h * F : (h + 1) * F], in_=bcps[h][:])

            # exp(s), exp(-s) broadcasts (bf16) for the VectorE chunks
            p_bc = cpool.tile([P, S], bf16, tag="pbc")
            nc.scalar.activation(out=p_bc[:], in_=s_bcast[:], func=Act.Exp, scale=1.0)
            pi_bc = cpool.tile([P, S], bf16, tag="pibc")
            nc.scalar.activation(out=pi_bc[:], in_=s_bcast[:], func=Act.Exp, scale=-1.0)

            # a few more warmup matmuls to keep HAM hot until real work lands
            wps = pspool.tile([P, F], f32, tag="ps", name="warmup_ps")
            for _ in range(6):
                nc.tensor.matmul(wps[:], ft_all[:, 0:P], ftc(0), start=True, stop=True)

            ogroups = [None] * (NT // OG)

            def epilogue(ps, m):
                g, slot = divmod(m, OG)
                if ogroups[g] is None:
                    ogroups[g] = opool.tile([P, OG * F], bf16, tag="og", name=f"og{g}")
                og = ogroups[g]
                nc.vector.tensor_scalar_mul(
                    og[:, slot * F : (slot + 1) * F], ps[:], pk[:, C_R + m : C_R + m + 1]
                )
                if slot == OG - 1:
                    dst = warped[g * OG * P : (g + 1) * OG * P, :].rearrange(
                        "(mm p) f -> p mm f", p=P
                    )
                    nc.sync.dma_start(
                        out=dst, in_=og[:].rearrange("p (mm f) -> p mm f", mm=OG)
                    )

            # Phase A: generate ET chunk-by-chunk; as each chunk lands, run
            # the 8 matmuls of output tiles 0..7 (one PSUM bank each).
            psA = [
                pspool.tile([P, F], f32, tag="ps", name=f"psA{m}") for m in range(GA)
            ]
            ets = []
            for c in range(NT):
                et = epool.tile([P, S], bf16, tag="et")
                if c in ACT_CHUNKS:
                    d = dpool.tile([P, S], f32, tag="d")
                    # d[p, i] = |s_i - t_{c*128+p}|  (ScalarE: Abs(in + bias))
                    nc.scalar.activation(
                        out=d[:],
                        in_=s_bcast[:],
                        func=Act.Abs,
                        bias=pk[:, C_NT + c : C_NT + c + 1],
                        scale=1.0,
                    )
                    nc.scalar.activation(out=et[:], in_=d[:], func=Act.Exp, scale=-1.0)
                else:
                    m1 = dpool.tile([P, S], bf16, tag="m1")
                    m2 = dpool.tile([P, S], bf16, tag="m2")
                    nc.vector.tensor_scalar_mul(
                        m1[:], p_bc[:], pk[:, C_Q + c : C_Q + c + 1]
                    )
                    nc.vector.tensor_scalar_mul(
                        m2[:], pi_bc[:], pk[:, C_QI + c : C_QI + c + 1]
                    )
                    nc.vector.tensor_tensor(out=et[:], in0=m1[:], in1=m2[:], op=Alu.min)
                ets.append(et)
                for m in range(GA):
                    nc.tensor.matmul(
                        psA[m][:],
                        et[:, m * P : (m + 1) * P],
                        ftc(c),
                        start=(c == 0),
                        stop=(c == NT - 1),
                    )
            for m in range(GA):
                epilogue(psA[m], m)

            # Phase B: all ET chunks resident; dense matmuls for tiles 8..15.
            for m in range(GA, NT):
                ps = pspool.tile([P, F], f32, tag="ps", name=f"psB{m}")
                for c in range(NT):
                    nc.tensor.matmul(
                        ps[:],
                        ets[c][:, m * P : (m + 1) * P],
                        ftc(c),
                        start=(c == 0),
                        stop=(c == NT - 1),
                    )
                epilogue(ps, m)

    nc.compile()
    return nc


def get_nc():
    if "nc" not in _NC_CACHE:
        _NC_CACHE["nc"] = _build_nc()
    return _NC_CACHE["nc"]


def _host_rden(score, template):
    """Exact softmax denominators: rden[b, i] = 1 / sum_j exp(-|s_bi - t_j|)."""
    s = score[:, :, 0].astype(np.float64)  # [B, S]
    t = np.sort(template[0, :, 0].astype(np.float64))  # [S]
    C = np.concatenate([[0.0], np.cumsum(np.exp(t))])  # C[k] = sum_{j<k} e^{t_j}
    D = np.concatenate([[0.0], np.cumsum(np.exp(-t)[::-1])])[::-1]  # sum_{j>=k} e^{-t}
    k = np.searchsorted(t, s.ravel(), side="right").reshape(s.shape)
    den = np.exp(-s) * C[k] + np.exp(s) * D[k]
    return (1.0 / den).astype(np.float32)  # [B, S]


def make_in_maps(score, feature, template):
    rden = _host_rden(score, template)
    s = np.ascontiguousarray(score[:, :, 0], dtype=np.float32)  # [B, S]
    t64 = template[0, :, 0].astype(np.float64)  # [S]
    bf = ml_dtypes.bfloat16

    def colmaj(v):  # [S] -> [128, 16] with v[c*128+p] at [p, c]
        return np.asarray(v, dtype=np.float32).reshape(NT, P).T

    nt_cols = colmaj(-t64)
    q_cols = colmaj(np.exp(-t64))
    qi_cols = colmaj(np.exp(t64))
    in_maps = []
    for b in range(B):
        pk = np.concatenate(
            [nt_cols, colmaj(rden[b]), q_cols, qi_cols], axis=1
        )  # [128, 64]
        in_maps.append(
            {
                "score": s[b],
                "packed": np.ascontiguousarray(pk),
                "feature": np.asarray(feature[b], dtype=np.float32).astype(bf),
            }
        )
    return in_maps


def postprocess(results):
    """results: per-core list of {name: np.ndarray} -> (warped, l2)."""
    warped = np.stack(
        [np.asarray(results[b]["warped"]).astype(np.float32) for b in range(B)]
    )
    l2 = np.float32(L2_REG_WEIGHT * np.sqrt(np.sum(warped.astype(np.float64) ** 2)))
    return warped, l2


def kernel(score, feature, template):
    from concourse.bass_utils import run_bass_kernel_spmd

    nc = get_nc()
    in_maps = make_in_maps(score, feature, template)
    res = run_bass_kernel_spmd(nc, in_maps, core_ids=list(range(NCORES)))
    return postprocess(res.results)


# revision 17
# speedup vs baseline: 1.0961x; 1.0556x over previous
"""Trainium2 Bass kernel for a differentiable-DTW style module.

Math (per batch b):
    dist[i, j] = |score[b, i] - template[j]|              (i, j in [0, 2048))
    path       = softmax(-dist, axis=-1)                  (row-stochastic)
    warped[b]  = path @ feature[b]                        ([2048, 512])
    l2         = 1e-7 * sqrt(sum over all b of warped ** 2)

Implementation notes:
  - Data-parallel: batch b -> NeuronCore b (B == 8 == n_cores).
  - Softmax row sums are computed EXACTLY on the host with a sorted
    prefix-sum identity:
        sum_j exp(-|s - t_j|) = exp(-s) * sum_{t_j <= s} exp(t_j)
                              + exp(s)  * sum_{t_j >  s} exp(-t_j)
    so the device only scales matmul output rows by 1/den.
  - Kernel matrix generated directly in TRANSPOSED layout
    ET[j, i] = exp(-|s_i - t_j|) (template on partitions) == the lhsT the
    tensor engine wants; feature [j, f] is the rhs.
  - ET is bf16; generation is split between ScalarE (Abs then Exp, with a
    per-partition -t bias) and VectorE (E = min(e^s e^-t, e^-s e^t), two
    tensor_scalar_mul + one tensor_tensor min, all bf16 fast modes).
  - score row is broadcast to 128 partitions with a tensor-engine outer
    product against ones (doubles as HAM warmup); exp(+-s) broadcasts are
    then one ScalarE Exp each.
  - DMAs are coalesced (one feature load, one packed-constant load, four
    grouped stores) because each dma_start costs ~1.2us of sequencer time.
  - Output leaves the device in bf16; host upcasts and finishes the l2.
"""

import numpy as np
import ml_dtypes

B = 8
S = 2048
F = 512
P = 128
NT = S // P  # 16 chunks / output tiles
NCORES = 8
GA = 8  # phase-A output-tile group (== number of PSUM banks)
OG = 4  # output m-tiles grouped per store DMA
L2_REG_WEIGHT = 1e-07

# packed[:, c] column map: 0:16 -t | 16:32 1/den | 32:48 e^-t | 48:64 e^t
C_NT, C_R, C_Q, C_QI = 0, NT, 2 * NT, 3 * NT

_NC_CACHE = {}


def _build_nc():
    import concourse.mybir as mybir
    import concourse.tile as tile
    from concourse import bacc

    f32 = mybir.dt.float32
    bf16 = mybir.dt.bfloat16
    Alu = mybir.AluOpType
    Act = mybir.ActivationFunctionType

    nc = bacc.Bacc(None, target_bir_lowering=False)
    score = nc.dram_tensor("score", [S], f32, kind="ExternalInput")
    packed = nc.dram_tensor("packed", [P, 4 * NT], f32, kind="ExternalInput")
    feat = nc.dram_tensor("feature", [S, F], bf16, kind="ExternalInput")
    warped = nc.dram_tensor("warped", [S, F], bf16, kind="ExternalOutput")

    # ScalarE-generated chunks (Abs+Exp); the rest on VectorE via
    # E = min(exp(s)exp(-t), exp(-s)exp(t))  (exactly exp(-|s-t|))
    ACT_CHUNKS = {0, 2, 4, 6, 8, 10, 12}

    with tile.TileContext(nc) as tc:
        with (
            tc.tile_pool(name="const", bufs=1) as cpool,
            tc.tile_pool(name="feat", bufs=1) as fpool,
            tc.tile_pool(name="et", bufs=NT) as epool,
            tc.tile_pool(name="dtile", bufs=3) as dpool,
            tc.tile_pool(name="otile", bufs=2) as opool,
            tc.tile_pool(name="ps", bufs=8, space="PSUM") as pspool,
        ):
            ones = cpool.tile([1, P], f32, tag="ones")
            nc.gpsimd.memset(ones[:], 1.0)
            scratch = cpool.tile([1, 1], f32, tag="scratch")
            # dummy activation: forces the ACT table set load at t~0
            nc.scalar.activation(out=scratch[:], in_=ones[:, 0:1], func=Act.Exp)

            # s_bcast via replicated-row DMA on the ACT HWDGE ring, first.
            s_bcast = cpool.tile([P, S], f32, tag="sb")
            nc.scalar.dma_start(
                out=s_bcast[:], in_=score[:].unsqueeze(0).to_broadcast([P, S])
            )
            pk = cpool.tile([P, 4 * NT], f32, tag="pk")
            nc.scalar.dma_start(out=pk[:], in_=packed[:])

            # feature: first chunk alone (unblocks warmup fast), rest coalesced
            ft0 = fpool.tile([P, F], bf16, tag="ft0")
            nc.sync.dma_start(out=ft0[:], in_=feat[0:P, :])
            ft_rest = fpool.tile([P, (NT - 1) * F], bf16, tag="ft")
            nc.sync.dma_start(
                out=ft_rest[:].rearrange("p (c f) -> p c f", c=NT - 1),
                in_=feat[P:, :].rearrange("(c p) f -> p c f", p=P),
            )

            def ftc(c):
                if c == 0:
                    return ft0[:]
                return ft_rest[:, (c - 1) * F : c * F]

            # warmup matmuls: keep HAM hot until real work lands
            wps = pspool.tile([P, F], f32, tag="ps", name="warmup_ps")
            for _ in range(10):
                nc.tensor.matmul(wps[:], ft0[:, 0:P], ft0[:], start=True, stop=True)

            # exp(s), exp(-s) broadcasts (bf16) for the VectorE chunks
            p_bc = cpool.tile([P, S], bf16, tag="pbc")
            nc.scalar.activation(out=p_bc[:], in_=s_bcast[:], func=Act.Exp, scale=1.0)
            pi_bc = cpool.tile([P, S], bf16, tag="pibc")
            nc.scalar.activation(out=pi_bc[:], in_=s_bcast[:], func=Act.Exp, scale=-1.0)

            ogroups = [None] * (NT // OG)

            def epilogue(ps, m):
                g, slot = divmod(m, OG)
                if ogroups[g] is None:
                    ogroups[g] = opool.tile([P, OG * F], bf16, tag="og", name=f"og{g}")
                og = ogroups[g]
                nc.vector.tensor_scalar_mul(
                    og[:, slot * F : (slot + 1) * F], ps[:], pk[:, C_R + m : C_R + m + 1]
                )
                if slot == OG - 1:
                    dst = warped[g * OG * P : (g + 1) * OG * P, :].rearrange(
                        "(mm p) f -> p mm f", p=P
                    )
                    nc.sync.dma_start(
                        out=dst, in_=og[:].rearrange("p (mm f) -> p mm f", mm=OG)
                    )

            # Phase A: generate ET chunk-by-chunk; as each chunk lands, run
            # the 8 matmuls of output tiles 0..7 (one PSUM bank each).
            psA = [
                pspool.tile([P, F], f32, tag="ps", name=f"psA{m}") for m in range(GA)
            ]
            ets = []
            for c in range(NT):
                et = epool.tile([P, S], bf16, tag="et")
                if c in ACT_CHUNKS:
                    d = dpool.tile([P, S], f32, tag="d")
                    # d[p, i] = |s_i - t_{c*128+p}|  (ScalarE: Abs(in + bias))
                    nc.scalar.activation(
                        out=d[:],
                        in_=s_bcast[:],
                        func=Act.Abs,
                        bias=pk[:, C_NT + c : C_NT + c + 1],
                        scale=1.0,
                    )
                    nc.scalar.activation(out=et[:], in_=d[:], func=Act.Exp, scale=-1.0)
                else:
                    m1 = dpool.tile([P, S], bf16, tag="m1")
                    m2 = dpool.tile([P, S], bf16, tag="m2")
                    nc.vector.tensor_scalar_mul(
                        m1[:], p_bc[:], pk[:, C_Q + c : C_Q + c + 1]
                    )
                    nc.vector.tensor_scalar_mul(
                        m2[:], pi_bc[:], pk[:, C_QI + c : C_QI + c + 1]
                    )
                    nc.vector.tensor_tensor(out=et[:], in0=m1[:], in1=m2[:], op=Alu.min)
                ets.append(et)
                for m in range(GA):
                    nc.tensor.matmul(
                        psA[m][:],
                        et[:, m * P : (m + 1) * P],
                        ftc(c),
                        start=(c == 0),
                        stop=(c == NT - 1),
                    )
            for m in range(GA):
                epilogue(psA[m], m)

            # Phase B: all ET chunks resident; dense matmuls for tiles 8..15.
            for m in range(GA, NT):
                ps = pspool.tile([P, F], f32, tag="ps", name=f"psB{m}")
                for c in range(NT):
                    nc.tensor.matmul(
                        ps[:],
                        ets[c][:, m * P : (m + 1) * P],
                        ftc(c),
                        start=(c == 0),
                        stop=(c == NT - 1),
                    )
                epilogue(ps, m)

    nc.compile()
    return nc


def get_nc():
    if "nc" not in _NC_CACHE:
        _NC_CACHE["nc"] = _build_nc()
    return _NC_CACHE["nc"]


def _host_rden(score, template):
    """Exact softmax denominators: rden[b, i] = 1 / sum_j exp(-|s_bi - t_j|)."""
    s = score[:, :, 0].astype(np.float64)  # [B, S]
    t = np.sort(template[0, :, 0].astype(np.float64))  # [S]
    C = np.concatenate([[0.0], np.cumsum(np.exp(t))])  # C[k] = sum_{j<k} e^{t_j}
    D = np.concatenate([[0.0], np.cumsum(np.exp(-t)[::-1])])[::-1]  # sum_{j>=k} e^{-t}
    k = np.searchsorted(t, s.ravel(), side="right").reshape(s.shape)
    den = np.exp(-s) * C[k] + np.exp(s) * D[k]
    return (1.0 / den).astype(np.float32)  # [B, S]


def make_in_maps(score, feature, template):
    rden = _host_rden(score, template)
    s = np.ascontiguousarray(score[:, :, 0], dtype=np.float32)  # [B, S]
    t64 = template[0, :, 0].astype(np.float64)  # [S]
    bf = ml_dtypes.bfloat16

    def colmaj(v):  # [S] -> [128, 16] with v[c*128+p] at [p, c]
        return np.asarray(v, dtype=np.float32).reshape(NT, P).T

    nt_cols = colmaj(-t64)
    q_cols = colmaj(np.exp(-t64))
    qi_cols = colmaj(np.exp(t64))
    in_maps = []
    for b in range(B):
        pk = np.concatenate(
            [nt_cols, colmaj(rden[b]), q_cols, qi_cols], axis=1
        )  # [128, 64]
        in_maps.append(
            {
                "score": s[b],
                "packed": np.ascontiguousarray(pk),
                "feature": np.asarray(feature[b], dtype=np.float32).astype(bf),
            }
        )
    return in_maps


def postprocess(results):
    """results: per-core list of {name: np.ndarray} -> (warped, l2)."""
    warped = np.stack(
        [np.asarray(results[b]["warped"]).astype(np.float32) for b in range(B)]
    )
    l2 = np.float32(L2_REG_WEIGHT * np.sqrt(np.sum(warped.astype(np.float64) ** 2)))
    return warped, l2


def kernel(score, feature, template):
    from concourse.bass_utils import run_bass_kernel_spmd

    nc = get_nc()
    in_maps = make_in_maps(score, feature, template)
    res = run_bass_kernel_spmd(nc, in_maps, core_ids=list(range(NCORES)))
    return postprocess(res.results)
